# revision 2
# baseline (speedup 1.0000x reference)
"""CTPN loss kernel for Trainium2 (Bass), data-parallel over 8 NeuronCores.

The loss touches 64 positive + 64 negative anchor positions of the
(1, 512, 1024, 50) score map. Rows (H=512) are sharded across 8 cores (64
rows, 13.1MB each). Host-side prep is limited to index translation and
constant folding (weights/masks derived from indices, targets and o_mask —
never from x); every read of x and all loss math happen on device.

Per core the device pipeline is a single lean dependency chain:
  SP:     DMA the per-anchor gather offsets [128,1] i32, then the packed
          per-anchor weighted select masks pk [128,208] f32
  gpsimd: indirect row gather G[128, 0:50] <- xs[flat[p]] (SWDGE), then aux
          memsets (G col 50 = 1.0, zero/one activation-bias columns) run in
          the gather's shadow
  DVE:    E = reduce(M (*) broadcast(G51)) -> [dv0, dv1, do, dcls] where M's
          blocks one-hot-select channels and carry -target in col 50
          smooth-l1 via s = h*(|d|-h), h = 0.5*min(|d|,1) (the 2x folds into
          the host weights)
  ACT:    CE = ln(1 + exp(dcls)) (exp+ln, single table load, off-path)
  DVE:    WF = [s0 s1 s2 CE] * W4;  PE: ones^T @ WF -> PSUM [1,4]
  DVE:    copy PSUM->SBUF;  SP: DMA out [1,4]
Host sums the 8 cores' [1,4] partials (the data-parallel all-reduce).

Same-engine RAW hazards are ordered with a per-engine counting semaphore
(engine pipelines do not interlock on SBUF read-after-write).
"""

import types

import numpy as np

import bass_rust as _bass_rust
import concourse.bacc as bacc
import concourse.bass as bass
import concourse.mybir as mybir
from concourse.bass_utils import run_bass_kernel_spmd
from concourse.hw_specs import get_activation_tables

H, W, C, K = 512, 1024, 50, 10
NP, NN = 64, 64
NA = NP + NN
NCORES = 8
HS = H // NCORES
ROWS = HS * W
CB = C + 1          # 51: select block width (col 50 = -target)
PKW = 4 * CB + 4    # 208: 204 mask cols + 4 weights

f32 = mybir.dt.float32
i32 = mybir.dt.int32
u32 = mybir.dt.uint32
Alu = mybir.AluOpType
Act = mybir.ActivationFunctionType

TRACE = False
LAST_RESULT = None
_NC_CACHE = None


def _patched_insert_act_table_loads(self):
    """Restrict the ACT-table chooser to natural_log_exp_and_others so Exp
    and Ln resolve to one table (one ACT_TABLE_LOAD instead of two)."""
    has_activation = any(
        isinstance(i, mybir.InstActivation)
        for b in self.main_func.blocks
        for i in b.instructions
    )
    if not has_activation:
        return
    tables = [
        (name, funcs if name == "natural_log_exp_and_others" else set())
        for name, funcs in get_activation_tables(self.m.arch).items()
    ]
    _bass_rust.insert_act_table_loads(self, tables)


def _strip_const_memsets(nc):
    """Drop the framework const-ap memsets (we pass explicit bias APs
    instead); they would otherwise be the first compute ops of the kernel."""
    blk = nc.main_func.blocks[0]
    dead = [
        inst
        for inst in blk.instructions
        if isinstance(inst, mybir.InstMemset) and "@const-" in inst.concise()
    ]
    for inst in dead:
        blk.instructions.remove(inst)
    assert len(dead) == 4, f"expected 4 const memsets, found {len(dead)}"


def _build_nc():
    nc = bacc.Bacc("TRN2", target_bir_lowering=False, debug=False)
    nc.insert_act_table_loads = types.MethodType(_patched_insert_act_table_loads, nc)
    _strip_const_memsets(nc)

    xs = nc.dram_tensor("xs", [ROWS, C], f32, kind="ExternalInput")
    off = nc.dram_tensor("off", [NA, 1], i32, kind="ExternalInput")
    pk = nc.dram_tensor("pk", [NA, PKW], f32, kind="ExternalInput")
    out = nc.dram_tensor("out", [1, 4], f32, kind="ExternalOutput")

    OFF = nc.alloc_sbuf_tensor("OFF", [NA, 1], i32)
    PK = nc.alloc_sbuf_tensor("PK", [NA, PKW], f32)
    G51 = nc.alloc_sbuf_tensor("G51", [NA, CB], f32)
    E = nc.alloc_sbuf_tensor("E", [NA, 4], f32)
    SEL = nc.alloc_sbuf_tensor("SEL", [NA, 4 * CB], f32)
    AH = nc.alloc_sbuf_tensor("AH", [NA, 6], f32)   # |d| (0:3), h (3:6)
    F = nc.alloc_sbuf_tensor("F", [NA, 4], f32)     # s0 s1 s2 CE
    EX = nc.alloc_sbuf_tensor("EX", [NA, 1], f32)
    WF = nc.alloc_sbuf_tensor("WF", [NA, 4], f32)
    ONES = nc.alloc_sbuf_tensor("ONES", [NA, 1], f32)
    ZERO = nc.alloc_sbuf_tensor("ZERO", [NA, 1], f32)
    O4 = nc.alloc_sbuf_tensor("O4", [1, 4], f32)
    P4 = nc.alloc_psum_tensor("P4", [1, 4], f32)

    sOFF = nc.alloc_semaphore("sOFF")
    sPK = nc.alloc_semaphore("sPK")
    sG = nc.alloc_semaphore("sG")
    sCE = nc.alloc_semaphore("sCE")
    sWF = nc.alloc_semaphore("sWF")
    sMM = nc.alloc_semaphore("sMM")
    sO4 = nc.alloc_semaphore("sO4")
    sOut = nc.alloc_semaphore("sOut")
    sIni = nc.alloc_semaphore("sIni")

    # ---- t0: offsets DMA first (tiny -> fastest completion), then masks ----
    nc.sync.dma_start(OFF.ap(), off.ap()).then_inc(sOFF, 16)
    nc.sync.dma_start(PK.ap(), pk.ap()).then_inc(sPK, 16)

    # ---- gather, issued before any compute op; memsets run in its shadow ----
    gi = nc.gpsimd.indirect_dma_start(
        out=G51.ap()[:, 0:C],
        out_offset=None,
        in_=xs.ap(),
        in_offset=bass.IndirectOffsetOnAxis(ap=OFF.ap(), axis=0),
    )
    gi._wait_ge(sOFF, 16)
    gi.then_inc(sG, 16)
    nc.gpsimd.memset(G51.ap()[:, C:CB], 1.0)
    nc.gpsimd.memset(ZERO.ap(), 0.0)
    nc.gpsimd.memset(ONES.ap(), 1.0).then_inc(sIni, 1)

    # ---- select + reduce (DVE); sV orders same-engine RAW hazards ----
    sV = nc.alloc_semaphore("sV")
    Mv = PK.ap()[:, 0:4 * CB].rearrange("p (b c) -> p b c", c=CB)
    Gb = G51.ap()[:, None, :].to_broadcast([NA, 4, CB])
    SELv = SEL.ap().rearrange("p (b c) -> p b c", c=CB)
    nc.vector.wait_ge(sPK, 16)
    nc.vector.wait_ge(sIni, 1)
    tt = nc.vector.tensor_tensor(SELv, Mv, Gb, op=Alu.mult)
    tt._wait_ge(sG, 16)
    tt.then_inc(sV, 1)
    rd = nc.vector.reduce_sum(E.ap()[:, :, None], SELv, axis=mybir.AxisListType.X)
    rd._wait_ge(sV, 1)
    rd.then_inc(sV, 1)

    # ---- CE on ACT: ln(1 + exp(dcls)) ----
    nc.scalar.wait_ge(sIni, 1)
    ae = nc.scalar.activation(EX.ap(), E.ap()[:, 3:4], Act.Exp,
                              bias=ZERO.ap()[:, 0:1])
    ae._wait_ge(sV, 2)
    ae.then_inc(sCE, 1)
    al = nc.scalar.activation(F.ap()[:, 3:4], EX.ap(), Act.Ln,
                              bias=ONES.ap()[:, 0:1])
    al._wait_ge(sCE, 1)
    al.then_inc(sCE, 1)

    # ---- smooth-l1 on DVE: s = h*(|d|-h), h = 0.5*min(|d|,1) ----
    t1 = nc.vector.tensor_scalar(AH.ap()[:, 0:3].bitcast(u32),
                                 E.ap()[:, 0:3].bitcast(u32),
                                 0x7FFFFFFF, None, op0=Alu.bitwise_and)
    t1._wait_ge(sV, 2)
    t1.then_inc(sV, 1)
    t2 = nc.vector.tensor_scalar(AH.ap()[:, 3:6], AH.ap()[:, 0:3], 1.0, 0.5,
                                 op0=Alu.min, op1=Alu.mult)
    t2._wait_ge(sV, 3)
    t2.then_inc(sV, 1)
    t3 = nc.vector.tensor_tensor(F.ap()[:, 0:3], AH.ap()[:, 0:3], AH.ap()[:, 3:6],
                                 op=Alu.subtract)
    t3._wait_ge(sV, 4)
    t3.then_inc(sV, 1)
    t4 = nc.vector.tensor_tensor(F.ap()[:, 0:3], F.ap()[:, 0:3], AH.ap()[:, 3:6],
                                 op=Alu.mult)
    t4._wait_ge(sV, 5)
    t4.then_inc(sV, 1)
    nc.vector.wait_ge(sCE, 2)
    wfi = nc.vector.tensor_tensor(WF.ap(), F.ap(), PK.ap()[:, 4 * CB:4 * CB + 4],
                                  op=Alu.mult)
    wfi._wait_ge(sV, 6)
    wfi.then_inc(sWF, 1)

    # ---- partition reduce on PE, copy out, DMA ----
    nc.tensor.wait_ge(sIni, 1)
    mm = nc.tensor.matmul(out=P4.ap(), lhsT=ONES.ap(), rhs=WF.ap(),
                          start=True, stop=True)
    mm._wait_ge(sWF, 1)
    mm.then_inc(sMM, 1)

    cp = nc.vector.tensor_copy(O4.ap(), P4.ap())
    cp._wait_ge(sMM, 1)
    cp.then_inc(sO4, 1)

    od = nc.sync.dma_start(out.ap(), O4.ap())
    od._wait_ge(sO4, 1)
    od.then_inc(sOut, 16)
    # No explicit completion wait: the NEFF exit sequence drains the DMA
    # queues before the engines retire, which fences the output write.

    nc.compile()
    return nc


def _get_nc():
    global _NC_CACHE
    if _NC_CACHE is None:
        _NC_CACHE = _build_nc()
    return _NC_CACHE


def make_in_maps(x, v_targets, o_targets, pos_y, pos_x, pos_z,
                 neg_y, neg_x, neg_z, o_mask):
    """Shard the inputs into per-core input maps. Host work: slice x by rows,
    translate anchor coords to each shard's local row-major layout, and fold
    the index/target/mask-derived constants into per-anchor weights and
    one-hot select masks. No math on x values happens on host."""
    xr = np.ascontiguousarray(x).reshape(H * W, C)
    pos_y = pos_y.astype(np.int64); pos_x = pos_x.astype(np.int64)
    pos_z = pos_z.astype(np.int64); neg_y = neg_y.astype(np.int64)
    neg_x = neg_x.astype(np.int64); neg_z = neg_z.astype(np.int64)
    vt = np.asarray(v_targets, np.float32)
    ot = np.asarray(o_targets, np.float32)
    om = np.asarray(o_mask).astype(np.float32)
    n_o = float(om.sum())

    ally = np.concatenate([pos_y, neg_y])
    allx = np.concatenate([pos_x, neg_x])
    allz = np.concatenate([pos_z, neg_z])
    ispos = np.zeros(NA, np.float32); ispos[:NP] = 1.0
    sigma = np.where(np.arange(NA) < NP, 1.0, -1.0).astype(np.float32)

    ch_v0 = 2 * allz
    ch_v1 = 2 * allz + 1
    ch_o = 4 * K + allz
    ch_c0 = 2 * K + 2 * allz
    ch_c1 = 2 * K + 2 * allz + 1

    M = np.zeros((NA, 4, CB), np.float32)
    ar = np.arange(NA)
    M[ar[:NP], 0, ch_v0[:NP]] = 1.0
    M[ar[:NP], 1, ch_v1[:NP]] = 1.0
    M[ar[:NP], 2, ch_o[:NP]] = 1.0
    M[ar[:NP], 0, C] = -vt[:, 0]
    M[ar[:NP], 1, C] = -vt[:, 1]
    M[ar[:NP], 2, C] = -ot
    M[ar, 3, ch_c0] = sigma
    M[ar, 3, ch_c1] = -sigma

    in_maps = []
    for i in range(NCORES):
        ly = ally - HS * i
        valid = ((ly >= 0) & (ly < HS)).astype(np.float32)
        flat = (np.clip(ly, 0, HS - 1) * W + allx).astype(np.int32)
        w4 = np.zeros((NA, 4), np.float32)
        # sl1 = 2*h*(|d|-h); the 2x and the reference's /2 (v) & /n_o scales
        # fold together here. CE weight = valid/128.
        w4[:, 0] = valid * ispos / n_o
        w4[:, 1] = valid * ispos / n_o
        w4[:, 2] = valid * ispos * 2.0 * np.concatenate(
            [om, np.zeros(NN, np.float32)]) / n_o
        w4[:, 3] = valid / NA
        pkbuf = np.zeros((NA, PKW), np.float32)
        pkbuf[:, 0:4 * CB] = M.reshape(NA, 4 * CB)
        pkbuf[:, 4 * CB:4 * CB + 4] = w4
        in_maps.append({
            "xs": xr[ROWS * i: ROWS * (i + 1)],
            "off": flat[:, None],
            "pk": pkbuf,
        })
    return in_maps


def kernel(**inputs):
    global LAST_RESULT
    nc = _get_nc()
    inputs = {k: np.asarray(v) for k, v in inputs.items()}
    in_maps = make_in_maps(**inputs)
    res = run_bass_kernel_spmd(nc, in_maps, core_ids=list(range(NCORES)), trace=TRACE)
    LAST_RESULT = res
    total = np.float64(0.0)
    for core_out in res.results:
        total += np.float64(core_out["out"].sum())
    return np.array(np.float32(total))
